# revision 6
# baseline (speedup 1.0000x reference)
"""Trainium2 Bass kernel for nn_LowRankLayer_dilation (B=4, C=64, H=W=128).

Math: the reference's rank-3 NMF update collapses exactly (all ranks are
initialized identically), and the eps terms are negligible for this input
distribution (denominators >= 0.2 everywhere vs eps=1e-6), giving:

    h   = relu(W_head @ x)            (per-pixel channel matmul)
    g   = W_tail @ h                  (per-pixel channel matmul)
    a   = box9(h)                     (3x3 dilation-2 box sum, edge-clamped)
    n_k = sum_c (a/9)_c * h_c(p+d_k)  (9 taps, d in {-2,0,2}^2)
    out = x + (n_4 / sum_j n_j^2) * sum_k n_k * g(p+d_k)

Sharding: pure data parallel, 8 cores = (batch b, H-half). Each core gets a
68-row halo'd slice packed as 2 channel blocks on 128 partitions:
partition p = c + 64*blk, blk A = slice rows 0..35, blk B = rows 32..67.
Channel reductions/broadcasts run on the PE via block-structured 0/1
matrices; pixel shifts are free-dim offset views (w-clamp via shifted
copies).
"""
import sys
import contextlib
import numpy as np

sys.path.insert(0, '/opt/trn_rl_repo')

import concourse.bass as bass  # noqa: E402,F401
import concourse.bacc as bacc  # noqa: E402
import concourse.tile as tile  # noqa: E402
import concourse.mybir as mybir  # noqa: E402
from concourse.bass_utils import run_bass_kernel_spmd  # noqa: E402

F32 = mybir.dt.float32
F32R = mybir.dt.float32r
BF16 = mybir.dt.bfloat16

N_CORES = 8
C = 64
RIN = 36          # per-block input rows (with +-2 halo)
ROUT = 32         # per-block output rows
W = 128
FIN = RIN * W     # 4608
FOUT = ROUT * W   # 4096
OFFS = [(di, dj) for di in (-2, 0, 2) for dj in (-2, 0, 2)]

EDT = BF16        # elementwise dtype for the inner NMF path


def _r3(ap):
    return ap.rearrange("p (r w) -> p r w", w=W)


def _shift_copy(nc, dst, src, dj):
    """dst[p, r, w] = src[p, r, clamp(w + dj)] for dj in {-2, +2}."""
    d3, s3 = _r3(dst), _r3(src)
    if dj == -2:
        nc.vector.tensor_copy(d3[:, :, 2:W], s3[:, :, 0:W - 2])
        nc.vector.tensor_copy(d3[:, :, 0:1], s3[:, :, 0:1])
        nc.vector.tensor_copy(d3[:, :, 1:2], s3[:, :, 0:1])
    else:
        nc.vector.tensor_copy(d3[:, :, 0:W - 2], s3[:, :, 2:W])
        nc.vector.tensor_copy(d3[:, :, W - 2:W - 1], s3[:, :, W - 1:W])
        nc.vector.tensor_copy(d3[:, :, W - 1:W], s3[:, :, W - 1:W])


def _build():
    nc = bacc.Bacc("TRN2", target_bir_lowering=False, debug=False,
                   num_devices=N_CORES)
    x_ext = nc.dram_tensor("x", [128, RIN, W], F32, kind="ExternalInput").ap()
    w2_ext = nc.dram_tensor("w2", [128, 128], F32, kind="ExternalInput").ap()
    w3_ext = nc.dram_tensor("w3", [128, 128], EDT, kind="ExternalInput").ap()
    bo_ext = nc.dram_tensor("bo", [128, 128], EDT, kind="ExternalInput").ap()
    sb_ext = nc.dram_tensor("sb", [18, 2], EDT, kind="ExternalInput").ap()
    bc2_ext = nc.dram_tensor("bc2", [2, 128], EDT, kind="ExternalInput").ap()
    y_ext = nc.dram_tensor("y", [128, ROUT, W], F32, kind="ExternalOutput").ap()

    with tile.TileContext(nc) as tc, contextlib.ExitStack() as ctx:
        cpool = ctx.enter_context(tc.tile_pool(name="consts", bufs=1))
        big = ctx.enter_context(tc.tile_pool(name="big", bufs=1))
        ppool = ctx.enter_context(tc.tile_pool(name="prod", bufs=2))
        npool = ctx.enter_context(tc.tile_pool(name="nbuf", bufs=2))
        psp = ctx.enter_context(tc.tile_pool(name="ps", bufs=2, space="PSUM"))

        w2 = cpool.tile([128, 128], F32)
        nc.sync.dma_start(w2[:], w2_ext[:])
        w3 = cpool.tile([128, 128], EDT)
        nc.sync.dma_start(w3[:], w3_ext[:])
        bo = cpool.tile([128, 128], EDT)
        nc.sync.dma_start(bo[:], bo_ext[:])
        sbm = cpool.tile([18, 2], EDT)
        nc.sync.dma_start(sbm[:], sb_ext[:])
        bc2 = cpool.tile([2, 128], EDT)
        nc.sync.dma_start(bc2[:], bc2_ext[:])

        xt = big.tile([128, FIN], F32)
        nc.sync.dma_start(_r3(xt)[:], x_ext[:])

        # ---- head: h = relu(W_head @ x), bf16 ----
        h = big.tile([128, FIN], EDT)
        for j in range(FIN // 512):
            ps = psp.tile([128, 512], F32, tag="mm1")
            nc.tensor.matmul(ps[:], w2[:],
                             xt[:, j * 512:(j + 1) * 512],
                             start=True, stop=True)
            nc.scalar.activation(h[:, j * 512:(j + 1) * 512], ps[:],
                                 mybir.ActivationFunctionType.Relu)

        # ---- g = W_tail @ h, bf16 ----
        g = big.tile([128, FIN], EDT)
        for j in range(FIN // 512):
            ps = psp.tile([128, 512], F32, tag="mm1")
            nc.tensor.matmul(ps[:], w3[:], h[:, j * 512:(j + 1) * 512],
                             start=True, stop=True)
            nc.scalar.copy(g[:, j * 512:(j + 1) * 512], ps[:])

        # ---- clamped w-shifts of h and g ----
        hm = big.tile([128, FIN], EDT)   # h at w-2
        hp = big.tile([128, FIN], EDT)   # h at w+2
        gm = big.tile([128, FIN], EDT)
        gp = big.tile([128, FIN], EDT)
        _shift_copy(nc, hm, h, -2)
        _shift_copy(nc, hp, h, +2)
        _shift_copy(nc, gm, g, -2)
        _shift_copy(nc, gp, g, +2)
        hS = {-2: hm, 0: h, 2: hp}
        gS = {-2: gm, 0: g, 2: gp}

        # ---- a = box9(h) (un-normalized; the 1/9 is folded into BO) ----
        t1 = big.tile([128, FIN], EDT, tag="scr")
        nc.vector.tensor_add(t1[:], h[:], hm[:])
        t2 = big.tile([128, FIN], EDT)
        nc.vector.tensor_add(t2[:], t1[:], hp[:])
        t2r = _r3(t2)
        a1 = big.tile([128, FOUT], EDT, tag="scr")
        a1r = a1.rearrange("p (r w) -> p r w", w=W)
        nc.vector.tensor_add(a1r[:], t2r[:, 0:ROUT, :], t2r[:, 2:ROUT + 2, :])
        av = big.tile([128, FOUT], EDT)
        avr = av.rearrange("p (r w) -> p r w", w=W)
        nc.vector.tensor_add(avr[:], a1r[:], t2r[:, 4:ROUT + 4, :])

        # ---- per-k: n_k (PE reduce+broadcast) and F accumulation ----
        CH = 1024
        nst = cpool.tile([18, FOUT], EDT)       # n_k rows, row = 2k+blk
        facc = big.tile([128, FOUT], EDT, tag="t2")  # reuse t2's slot
        for k, (di, dj) in enumerate(OFFS):
            hsrc = _r3(hS[dj])[:, 2 + di:2 + di + ROUT, :]
            prod = ppool.tile([128, FOUT], EDT, tag="pp")
            nc.vector.tensor_mul(_r3(prod)[:], avr[:], hsrc)
            nb = npool.tile([128, FOUT], EDT, tag="nb")
            for ch in range(FOUT // CH):
                pst = psp.tile([128, CH], F32, tag="nk")
                for q in range(CH // 512):
                    c0 = q * 512
                    nc.tensor.matmul(
                        pst[:, c0:c0 + 512], bo[:],
                        prod[:, ch * CH + c0:ch * CH + c0 + 512],
                        start=True, stop=True)
                nc.scalar.copy(nb[:, ch * CH:(ch + 1) * CH], pst[:])
            kr = (k - 4) % 9          # put k=4 (center) at rows 0..1
            nc.sync.dma_start(nst[2 * kr:2 * kr + 1, :], nb[0:1, :])
            nc.sync.dma_start(nst[2 * kr + 1:2 * kr + 2, :], nb[64:65, :])
            gsrc = _r3(gS[dj])[:, 2 + di:2 + di + ROUT, :]
            pk = ppool.tile([128, FOUT], EDT, tag="pp")
            nc.vector.tensor_mul(_r3(pk)[:], _r3(nb)[:], gsrc)
            if k == 0:
                nc.vector.tensor_copy(facc[:], pk[:])
            else:
                nc.vector.tensor_add(facc[:], facc[:], pk[:])

        # ---- Cf = n_4 / sum_j n_j^2, broadcast to 128 partitions ----
        nsq = npool.tile([18, FOUT], EDT, tag="nb")  # nb slots are free now
        nc.vector.tensor_mul(nsq[:], nst[:], nst[:])
        sn2 = cpool.tile([2, FOUT], F32)
        for j in range(FOUT // 512):
            ps2 = psp.tile([2, 512], F32, tag="s2")
            nc.tensor.matmul(ps2[:], sbm[:],
                             nsq[:, j * 512:(j + 1) * 512],
                             start=True, stop=True)
            nc.scalar.copy(sn2[:, j * 512:(j + 1) * 512], ps2[:])
        rcp = cpool.tile([2, FOUT], F32)
        nc.vector.reciprocal(rcp[:], sn2[:])
        cfr = cpool.tile([2, FOUT], EDT)
        nc.vector.tensor_mul(cfr[:], nst[0:2, :], rcp[:])
        cfb = big.tile([128, FOUT], EDT, tag="scr")  # a1/t1 are dead
        for ch in range(FOUT // CH):
            pst = psp.tile([128, CH], F32, tag="nk")
            for q in range(CH // 512):
                c0 = q * 512
                nc.tensor.matmul(pst[:, c0:c0 + 512], bc2[:],
                                 cfr[:, ch * CH + c0:ch * CH + c0 + 512],
                                 start=True, stop=True)
            nc.scalar.copy(cfb[:, ch * CH:(ch + 1) * CH], pst[:])

        # ---- out = x + Cf * F ----
        res = big.tile([128, FOUT], F32)
        nc.vector.tensor_mul(res[:], facc[:], cfb[:])
        xv = _r3(xt)[:, 2:2 + ROUT, :]
        nc.vector.tensor_add(_r3(res)[:], _r3(res)[:], xv)
        nc.sync.dma_start(y_ext[:], _r3(res)[:])

    nc.compile()
    return nc


_NC_CACHE = [None]


def _get_nc():
    if _NC_CACHE[0] is None:
        _NC_CACHE[0] = _build()
    return _NC_CACHE[0]


def _host_prep(x):
    B, Cc, H, Ww = x.shape
    in_maps = []
    for core in range(N_CORES):
        b, half = core // 2, core % 2
        r0 = 64 * half
        gidx = np.clip(np.arange(r0 - 2, r0 + 66), 0, H - 1)
        xs = x[b][:, gidx, :]                     # (64, 68, 128)
        packed = np.concatenate([xs[:, 0:36], xs[:, 32:68]], axis=0)
        in_maps.append({"x": np.ascontiguousarray(packed, np.float32)})
    return in_maps


def _const_maps(W_head, W_tail):
    import ml_dtypes

    def to_edt(a):
        return a.astype(ml_dtypes.bfloat16) if EDT == BF16 else a.astype(np.float32)

    w2 = np.zeros((128, 128), np.float32)
    w2[:64, :64] = W_head.T
    w2[64:, 64:] = W_head.T
    w3 = np.zeros((128, 128), np.float32)
    w3[:64, :64] = W_tail.T
    w3[64:, 64:] = W_tail.T
    bo = np.zeros((128, 128), np.float32)
    bo[:64, :64] = 1.0 / 9.0
    bo[64:, 64:] = 1.0 / 9.0
    sb = np.zeros((18, 2), np.float32)
    sb[0::2, 0] = 1.0
    sb[1::2, 1] = 1.0
    bc2 = np.zeros((2, 128), np.float32)
    bc2[0, :64] = 1.0
    bc2[1, 64:] = 1.0
    return {"w2": w2, "w3": to_edt(w3), "bo": to_edt(bo), "sb": to_edt(sb),
            "bc2": to_edt(bc2)}


def kernel(x, W_head, W_tail):
    x = np.asarray(x, np.float32)
    W_head = np.asarray(W_head, np.float32)
    W_tail = np.asarray(W_tail, np.float32)
    nc = _get_nc()
    consts = _const_maps(W_head, W_tail)
    in_maps = [{**m, **consts} for m in _host_prep(x)]
    res = run_bass_kernel_spmd(nc, in_maps, list(range(N_CORES)))
    out = np.empty_like(x)
    for core in range(N_CORES):
        b, half = core // 2, core % 2
        r0 = 64 * half
        y = res.results[core]["y"]               # (128, 32, 128)
        out[b, :, r0:r0 + 32, :] = y[:64]
        out[b, :, r0 + 32:r0 + 64, :] = y[64:]
    return out
